# revision 3
# baseline (speedup 1.0000x reference)
"""Trainium2 Bass kernel for nn_AttentionBlock (GroupNorm + 1x1-conv QKV +
full self-attention over N=HW=4096 + output projection + residual).

Distribution: data-parallel over batch B=8, one batch element per NeuronCore.

Per-core layout / algorithm (C=128 channels on SBUF partitions, N=4096 free):
  1. GroupNorm stats via two ACT passes (Square + Identity, both with
     accum_out row-sums), cross-partition group combine via tiny indicator
     matmuls on the PE.
  2. hn = a_c * x + b_c  (one ACT pass, output rounded to fp32r).
  3. Q, K in natural [c, n] layout (lhsT = host-pretransposed weights);
     V^T in [n, c] tile-major layout (lhsT = hn tiles). All fp32r.
  4. Main loop (2 halves x 32 j-tiles): S^T tile = K_j^T Q  (PE, fp32r),
     P^T = exp(S^T) (ACT, -> fp32r), acc += P^T (DVE, fp32 denominator
     partials), O += V^T_j^T P^T (PE accumulate in PSUM).
     No max-subtraction: logits are ~N(0,1) so exp is safe in fp32.
  5. rowsum = ones^T acc (PE), recip = 1/rowsum, broadcast via K=1 outer
     product matmul, O_norm = O * recip (DVE), proj = w_proj^T O_norm (PE),
     out = (x + b_eff) + proj, streamed to DRAM per 512-block.

Bias algebra: b_q, b_k folded into the Q/K PSUM->SBUF copies (ACT bias);
b_v folded into b_eff = b_proj + w_proj @ b_v (host precompute, exact).
The attention scale C^-0.5 is folded into w_q/b_q on the host (exact
linear-map reparameterization).
"""

import numpy as np

B, C, H, W = 8, 128, 64, 64
HW = H * W                      # 4096
GROUPS = 8
GSIZE = C // GROUPS             # 16
EPS = 1e-5
NJ = HW // 128                  # 32 j-tiles
IBLK = 512
NIB = HW // IBLK                # 8 i-blocks
NHALF = 2
HWID = HW // NHALF              # 2048
SCALE = float(C) ** -0.5

_CACHE = {}


def _build():
    from contextlib import ExitStack

    import concourse.bacc as bacc
    import concourse.tile as tile
    from concourse import mybir

    f32 = mybir.dt.float32
    f32r = mybir.dt.float32r
    AF = mybir.ActivationFunctionType

    nc = bacc.Bacc("TRN2", target_bir_lowering=False, debug=False)

    x_in = nc.dram_tensor("x", [C, HW], f32, kind="ExternalInput")
    gamma_in = nc.dram_tensor("gamma", [C, 1], f32, kind="ExternalInput")
    beta_in = nc.dram_tensor("beta", [C, 1], f32, kind="ExternalInput")
    bq_in = nc.dram_tensor("bq", [C, 1], f32, kind="ExternalInput")
    bk_in = nc.dram_tensor("bk", [C, 1], f32, kind="ExternalInput")
    beff_in = nc.dram_tensor("beff", [C, 1], f32, kind="ExternalInput")
    wq_in = nc.dram_tensor("wqT", [C, C], f32, kind="ExternalInput")
    wk_in = nc.dram_tensor("wkT", [C, C], f32, kind="ExternalInput")
    wv_in = nc.dram_tensor("wvT2", [C, 2 * C], f32, kind="ExternalInput")
    wp_in = nc.dram_tensor("wpT", [C, C], f32, kind="ExternalInput")
    ig_in = nc.dram_tensor("ig", [C, GROUPS], f32, kind="ExternalInput")
    igt_in = nc.dram_tensor("igt", [GROUPS, C], f32, kind="ExternalInput")
    out_dram = nc.dram_tensor("out", [C, HW], f32, kind="ExternalOutput")

    with tile.TileContext(nc) as tc, ExitStack() as ctx:
        const = ctx.enter_context(tc.tile_pool(name="const", bufs=1))
        big = ctx.enter_context(tc.tile_pool(name="big", bufs=1))
        stats = ctx.enter_context(tc.tile_pool(name="stats", bufs=1))
        ptpool = ctx.enter_context(tc.tile_pool(name="pt", bufs=3))
        row = ctx.enter_context(tc.tile_pool(name="row", bufs=1))

        # ---------------- constants ----------------
        def cload(t_in, shape, tag):
            t = const.tile(shape, f32, tag=tag)
            nc.sync.dma_start(t[:], t_in[:])
            return t

        gamma = cload(gamma_in, [C, 1], "c_gamma")
        beta = cload(beta_in, [C, 1], "c_beta")
        bq = cload(bq_in, [C, 1], "c_bq")
        bk = cload(bk_in, [C, 1], "c_bk")
        beff = cload(beff_in, [C, 1], "c_beff")
        ig = cload(ig_in, [C, GROUPS], "c_ig")
        igt = cload(igt_in, [GROUPS, C], "c_igt")
        wq_f = cload(wq_in, [C, C], "c_wq_f")
        wk_f = cload(wk_in, [C, C], "c_wk_f")
        wv_f = cload(wv_in, [C, 2 * C], "c_wv_f")
        wp_f = cload(wp_in, [C, C], "c_wp_f")

        wq = const.tile([C, C], f32r)
        nc.vector.tensor_copy(wq[:], wq_f[:])
        wk = const.tile([C, C], f32r)
        nc.vector.tensor_copy(wk[:], wk_f[:])
        wv2 = const.tile([C, 2 * C], f32r)
        nc.vector.tensor_copy(wv2[:], wv_f[:])
        wp = const.tile([C, C], f32r)
        nc.vector.tensor_copy(wp[:], wp_f[:])

        ones_c = const.tile([C, 1], f32)
        nc.vector.memset(ones_c[:], 1.0)
        ones_r = const.tile([1, C], f32r)
        nc.vector.memset(ones_r[:].bitcast(f32), 1.0)
        eps_t = const.tile([GROUPS, 1], f32)
        nc.vector.memset(eps_t[:], EPS)

        # ---------------- load x ----------------
        x_sb = big.tile([C, HW], f32, tag="x")
        nc.sync.dma_start(x_sb[:], x_in[:])

        # ---------------- groupnorm stats ----------------
        st2 = stats.tile([C, 2], f32)
        xsq = big.tile([C, HW], f32, tag="scratch")
        nc.scalar.activation(xsq[:], x_sb[:], AF.Square, accum_out=st2[:, 1:2])
        xres = big.tile([C, HW], f32, tag="xres")
        s1p = stats.tile([C, 1], f32)
        nc.scalar.activation(
            xres[:], x_sb[:], AF.Identity, bias=beff[:], accum_out=s1p[:]
        )
        tmpc = stats.tile([C, 1], f32)
        nc.vector.tensor_scalar_mul(tmpc[:], beff[:], float(HW))
        nc.vector.tensor_sub(st2[:, 0:1], s1p[:], tmpc[:])

        with tc.tile_pool(name="pa", bufs=2, space="PSUM") as pa:
            gs_ps = pa.tile([GROUPS, 2], f32, tag="qk")
            nc.tensor.matmul(gs_ps[:], ig[:], st2[:], start=True, stop=True)
            gstats = stats.tile([GROUPS, 2], f32)
            nc.vector.tensor_copy(gstats[:], gs_ps[:])
            inv_n = 1.0 / float(GSIZE * HW)
            gmean = stats.tile([GROUPS, 1], f32)
            nc.vector.tensor_scalar_mul(gmean[:], gstats[:, 0:1], inv_n)
            gm2 = stats.tile([GROUPS, 1], f32)
            nc.vector.tensor_scalar_mul(gm2[:], gstats[:, 1:2], inv_n)
            gmsq = stats.tile([GROUPS, 1], f32)
            nc.vector.tensor_mul(gmsq[:], gmean[:], gmean[:])
            gvar = stats.tile([GROUPS, 1], f32)
            nc.vector.tensor_sub(gvar[:], gm2[:], gmsq[:])
            gsd = stats.tile([GROUPS, 1], f32)
            nc.scalar.activation(gsd[:], gvar[:], AF.Sqrt, bias=eps_t[:])
            gmr = stats.tile([GROUPS, 2], f32)
            nc.vector.reciprocal(gmr[:, 1:2], gsd[:])
            nc.vector.tensor_copy(gmr[:, 0:1], gmean[:])

            bc_ps = pa.tile([C, 2], f32, tag="qk")
            nc.tensor.matmul(bc_ps[:], igt[:], gmr[:], start=True, stop=True)
            a_c = stats.tile([C, 1], f32)
            b_c = stats.tile([C, 1], f32)
            tmc = stats.tile([C, 1], f32)
            nc.vector.tensor_scalar_mul(a_c[:], gamma[:], bc_ps[:, 1:2])
            nc.vector.tensor_scalar_mul(tmc[:], a_c[:], bc_ps[:, 0:1])
            nc.vector.tensor_sub(b_c[:], beta[:], tmc[:])

            # hn = a_c * x + b_c   (fp32r)
            hn = big.tile([C, HW], f32r, tag="hn")
            nc.scalar.activation(hn[:], x_sb[:], AF.Identity, bias=b_c[:], scale=a_c[:])

            # ---------------- QKV ----------------
            q_r = big.tile([C, HW], f32r, tag="q")
            k_r = big.tile([C, HW], f32r, tag="k")
            vt = big.tile([C, NJ, C], f32r, tag="vt")
            for h in range(NHALF):
                qp = pa.tile([C, HWID], f32, tag="qk")
                for kk in range(4):
                    sl = slice(kk * IBLK, (kk + 1) * IBLK)
                    nc.tensor.matmul(
                        qp[:, sl], wq[:], hn[:, h * HWID + kk * IBLK:h * HWID + (kk + 1) * IBLK],
                        start=True, stop=True,
                    )
                nc.scalar.activation(
                    q_r[:, h * HWID:(h + 1) * HWID], qp[:], AF.Identity, bias=bq[:]
                )
            for h in range(NHALF):
                kp = pa.tile([C, HWID], f32, tag="qk")
                for kk in range(4):
                    sl = slice(kk * IBLK, (kk + 1) * IBLK)
                    nc.tensor.matmul(
                        kp[:, sl], wk[:], hn[:, h * HWID + kk * IBLK:h * HWID + (kk + 1) * IBLK],
                        start=True, stop=True,
                    )
                nc.scalar.activation(
                    k_r[:, h * HWID:(h + 1) * HWID], kp[:], AF.Identity, bias=bk[:]
                )
            for r in range(4):
                vp = pa.tile([C, 8, 2 * C], f32, tag="qk")
                for t in range(8):
                    nt = r * 8 + t
                    nc.tensor.matmul(
                        vp[:, t, :], hn[:, nt * 128:(nt + 1) * 128], wv2[:],
                        start=True, stop=True,
                    )
                nc.vector.tensor_copy(vt[:, r * 8:(r + 1) * 8, :], vp[:, :, 0:C])

        # ---------------- main attention loop ----------------
        acc = big.tile([C, HW], f32, tag="x")  # reuses x slot (x is dead)
        o_sb = big.tile([C, HW], f32, tag="o")
        with tc.tile_pool(name="ps_s", bufs=1, space="PSUM") as ps_s, \
             tc.tile_pool(name="ps_o", bufs=1, space="PSUM") as ps_o:
            for h in range(NHALF):
                hsl = slice(h * HWID, (h + 1) * HWID)
                op = ps_o.tile([C, HWID], f32)
                for j in range(NJ):
                    sp = ps_s.tile([C, HWID], f32)
                    for kk in range(4):
                        sl = slice(kk * IBLK, (kk + 1) * IBLK)
                        nc.tensor.matmul(
                            sp[:, sl],
                            k_r[:, j * 128:(j + 1) * 128],
                            q_r[:, h * HWID + kk * IBLK:h * HWID + (kk + 1) * IBLK],
                            start=True, stop=True,
                        )
                    pt = ptpool.tile([C, HWID], f32r)
                    nc.scalar.activation(pt[:], sp[:], AF.Exp)
                    if j == 0:
                        nc.vector.tensor_copy(acc[:, hsl], pt[:].bitcast(f32))
                    else:
                        nc.vector.tensor_add(acc[:, hsl], acc[:, hsl], pt[:].bitcast(f32))
                    for kk in range(4):
                        sl = slice(kk * IBLK, (kk + 1) * IBLK)
                        nc.tensor.matmul(
                            op[:, sl], vt[:, j, :], pt[:, sl],
                            start=(j == 0), stop=(j == NJ - 1),
                        )
                nc.scalar.activation(o_sb[:, hsl], op[:], AF.Copy)

        # ---------------- softmax denominators + projection ----------------
        recip = row.tile([1, HW], f32r)
        with tc.tile_pool(name="prs", bufs=2, space="PSUM") as prs:
            for ib in range(NIB):
                sl = slice(ib * IBLK, (ib + 1) * IBLK)
                rp = prs.tile([1, IBLK], f32)
                nc.tensor.matmul(rp[:], ones_c[:], acc[:, sl], start=True, stop=True)
                with nc.allow_low_precision(reason="fp32r rounding of softmax recip is benign"):
                    nc.vector.reciprocal(recip[:, sl], rp[:])

        o_nrm = big.tile([C, HW], f32r, tag="hn")  # reuses hn slot
        with tc.tile_pool(name="pbc", bufs=2, space="PSUM") as pbc:
            for ib in range(NIB):
                sl = slice(ib * IBLK, (ib + 1) * IBLK)
                bp = pbc.tile([C, IBLK], f32)
                nc.tensor.matmul(bp[:], ones_r[:], recip[:, sl], start=True, stop=True)
                nc.vector.tensor_mul(o_nrm[:, sl], o_sb[:, sl], bp[:])

        out_sb = big.tile([C, HW], f32, tag="scratch")  # reuses xsq slot
        with tc.tile_pool(name="ppj", bufs=2, space="PSUM") as ppj:
            for ib in range(NIB):
                sl = slice(ib * IBLK, (ib + 1) * IBLK)
                pp = ppj.tile([C, IBLK], f32)
                nc.tensor.matmul(pp[:], wp[:], o_nrm[:, sl], start=True, stop=True)
                nc.vector.tensor_add(out_sb[:, sl], xres[:, sl], pp[:])
                nc.sync.dma_start(out_dram[:, sl], out_sb[:, sl])

    nc.compile()
    return nc


def _get_nc():
    if "nc" not in _CACHE:
        _CACHE["nc"] = _build()
    return _CACHE["nc"]


def _prep_inputs(x, gamma, beta, w_qkv, b_qkv, w_proj, b_proj):
    x = np.ascontiguousarray(x, dtype=np.float32)
    w_qkv = np.asarray(w_qkv, dtype=np.float32)
    b_qkv = np.asarray(b_qkv, dtype=np.float32)
    w_proj = np.asarray(w_proj, dtype=np.float32)
    b_proj = np.asarray(b_proj, dtype=np.float32)

    wq = w_qkv[0:C, :]
    wk = w_qkv[C:2 * C, :]
    wv = w_qkv[2 * C:3 * C, :]
    bqv = b_qkv[0:C]
    bkv = b_qkv[C:2 * C]
    bvv = b_qkv[2 * C:3 * C]

    wqT = np.ascontiguousarray((wq * SCALE).T)
    wkT = np.ascontiguousarray(wk.T)
    wvT2 = np.concatenate(
        [np.ascontiguousarray(wv.T), np.zeros((C, C), np.float32)], axis=1
    )
    wpT = np.ascontiguousarray(w_proj.T)
    beff = (b_proj + w_proj @ bvv).astype(np.float32)

    ig = np.zeros((C, GROUPS), np.float32)
    ig[np.arange(C), np.arange(C) // GSIZE] = 1.0
    igt = np.ascontiguousarray(ig.T)

    common = {
        "gamma": np.asarray(gamma, np.float32).reshape(C, 1),
        "beta": np.asarray(beta, np.float32).reshape(C, 1),
        "bq": (bqv * SCALE).reshape(C, 1),
        "bk": bkv.reshape(C, 1),
        "beff": beff.reshape(C, 1),
        "wqT": wqT,
        "wkT": wkT,
        "wvT2": np.ascontiguousarray(wvT2),
        "wpT": wpT,
        "ig": ig,
        "igt": igt,
    }
    in_maps = []
    for b in range(B):
        m = dict(common)
        m["x"] = np.ascontiguousarray(x[b].reshape(C, HW))
        in_maps.append(m)
    return in_maps


def kernel(x, gamma, beta, w_qkv, b_qkv, w_proj, b_proj):
    from concourse.bass_utils import run_bass_kernel_spmd

    nc = _get_nc()
    in_maps = _prep_inputs(x, gamma, beta, w_qkv, b_qkv, w_proj, b_proj)
    res = run_bass_kernel_spmd(nc, in_maps, list(range(B)))
    out = np.stack([res.results[b]["out"] for b in range(B)], axis=0)
    return out.reshape(B, C, H, W).astype(np.float32)


# revision 6
# speedup vs baseline: 1.7258x; 1.7258x over previous
"""Trainium2 Bass kernel for nn_AttentionBlock (GroupNorm + 1x1-conv QKV +
full self-attention over N=HW=4096 + output projection + residual).

Distribution: data-parallel over batch B=8, one batch element per NeuronCore.

Per-core layout / algorithm (C=128 channels on SBUF partitions, N=4096 free):
  1. GroupNorm stats via two ACT passes (Square + Identity, both with
     accum_out row-sums), cross-partition group combine via tiny indicator
     matmuls on the PE.
  2. hn = a_c * x + b_c  (one ACT pass, output rounded to fp32r).
  3. Q, K in natural [c, n] layout (lhsT = host-pretransposed weights);
     V^T in [n, c] tile-major layout (lhsT = hn tiles). All fp32r.
  4. Main loop (2 halves x 32 j-tiles): S^T tile = K_j^T Q  (PE, fp32r),
     P^T = exp(S^T) (ACT, -> fp32r), acc += P^T (DVE, fp32 denominator
     partials), O += V^T_j^T P^T (PE accumulate in PSUM).
     No max-subtraction: logits are ~N(0,1) so exp is safe in fp32.
  5. rowsum = ones^T acc (PE), recip = 1/rowsum, broadcast via K=1 outer
     product matmul, O_norm = O * recip (DVE), proj = w_proj^T O_norm (PE),
     out = (x + b_eff) + proj, streamed to DRAM per 512-block.

Bias algebra: b_q, b_k folded into the Q/K PSUM->SBUF copies (ACT bias);
b_v folded into b_eff = b_proj + w_proj @ b_v (host precompute, exact).
The attention scale C^-0.5 is folded into w_q/b_q on the host (exact
linear-map reparameterization).
"""

import numpy as np

B, C, H, W = 8, 128, 64, 64
HW = H * W                      # 4096
GROUPS = 8
GSIZE = C // GROUPS             # 16
EPS = 1e-5
NJ = HW // 128                  # 32 j-tiles
IBLK = 512
NIB = HW // IBLK                # 8 i-blocks
NHALF = 2
HWID = HW // NHALF              # 2048
SCALE = float(C) ** -0.5

_CACHE = {}


def _build():
    from contextlib import ExitStack

    import concourse.bacc as bacc
    import concourse.tile as tile
    from concourse import mybir

    f32 = mybir.dt.float32
    f32r = mybir.dt.float32r
    AF = mybir.ActivationFunctionType

    nc = bacc.Bacc("TRN2", target_bir_lowering=False, debug=False)

    x_in = nc.dram_tensor("x", [C, HW], f32, kind="ExternalInput")
    gamma_in = nc.dram_tensor("gamma", [C, 1], f32, kind="ExternalInput")
    beta_in = nc.dram_tensor("beta", [C, 1], f32, kind="ExternalInput")
    bq_in = nc.dram_tensor("bq", [C, 1], f32, kind="ExternalInput")
    bk_in = nc.dram_tensor("bk", [C, 1], f32, kind="ExternalInput")
    beff_in = nc.dram_tensor("beff", [C, 1], f32, kind="ExternalInput")
    wq_in = nc.dram_tensor("wqT", [C, C], f32, kind="ExternalInput")
    wk_in = nc.dram_tensor("wkT", [C, C], f32, kind="ExternalInput")
    wv_in = nc.dram_tensor("wvT2", [C, 2 * C], f32, kind="ExternalInput")
    wp_in = nc.dram_tensor("wpT", [C, C], f32, kind="ExternalInput")
    ig_in = nc.dram_tensor("ig", [C, GROUPS], f32, kind="ExternalInput")
    igt_in = nc.dram_tensor("igt", [GROUPS, C], f32, kind="ExternalInput")
    out_dram = nc.dram_tensor("out", [C, HW], f32, kind="ExternalOutput")

    with tile.TileContext(nc) as tc, ExitStack() as ctx:
        const = ctx.enter_context(tc.tile_pool(name="const", bufs=1))
        big = ctx.enter_context(tc.tile_pool(name="big", bufs=1))
        stats = ctx.enter_context(tc.tile_pool(name="stats", bufs=1))
        ptpool = ctx.enter_context(tc.tile_pool(name="pt", bufs=3))
        row = ctx.enter_context(tc.tile_pool(name="row", bufs=1))

        # ---------------- constants ----------------
        def cload(t_in, shape, tag):
            t = const.tile(shape, f32, tag=tag)
            nc.sync.dma_start(t[:], t_in[:])
            return t

        gamma = cload(gamma_in, [C, 1], "c_gamma")
        beta = cload(beta_in, [C, 1], "c_beta")
        bq = cload(bq_in, [C, 1], "c_bq")
        bk = cload(bk_in, [C, 1], "c_bk")
        beff = cload(beff_in, [C, 1], "c_beff")
        ig = cload(ig_in, [C, GROUPS], "c_ig")
        igt = cload(igt_in, [GROUPS, C], "c_igt")
        wq_f = cload(wq_in, [C, C], "c_wq_f")
        wk_f = cload(wk_in, [C, C], "c_wk_f")
        wv_f = cload(wv_in, [C, 2 * C], "c_wv_f")
        wp_f = cload(wp_in, [C, C], "c_wp_f")

        wq = const.tile([C, C], f32r)
        nc.vector.tensor_copy(wq[:], wq_f[:])
        wk = const.tile([C, C], f32r)
        nc.vector.tensor_copy(wk[:], wk_f[:])
        wv2 = const.tile([C, 2 * C], f32r)
        nc.vector.tensor_copy(wv2[:], wv_f[:])
        wp = const.tile([C, C], f32r)
        nc.vector.tensor_copy(wp[:], wp_f[:])

        ones_c = const.tile([C, 1], f32)
        nc.vector.memset(ones_c[:], 1.0)
        ones_r = const.tile([1, C], f32r)
        nc.vector.memset(ones_r[:].bitcast(f32), 1.0)
        eps_t = const.tile([GROUPS, 1], f32)
        nc.vector.memset(eps_t[:], EPS)

        # ---------------- load x (chunked, stats overlap the DMA) ----------------
        NCH = 4
        CHW = HW // NCH
        x_sb = big.tile([C, HW], f32, tag="x")
        for ch in range(NCH):
            sl = slice(ch * CHW, (ch + 1) * CHW)
            nc.sync.dma_start(x_sb[:, sl], x_in[:, sl])

        st2 = stats.tile([C, 2], f32)
        s2p = stats.tile([C, NCH], f32)
        s1p = stats.tile([C, NCH], f32)
        xsq = big.tile([C, HW], f32, tag="scratch")
        xres = big.tile([C, HW], f32, tag="xres")
        for ch in range(NCH):
            sl = slice(ch * CHW, (ch + 1) * CHW)
            nc.scalar.activation(
                xsq[:, sl], x_sb[:, sl], AF.Square, accum_out=s2p[:, ch:ch + 1]
            )
            nc.scalar.activation(
                xres[:, sl], x_sb[:, sl], AF.Identity, bias=beff[:],
                accum_out=s1p[:, ch:ch + 1],
            )
        nc.vector.reduce_sum(st2[:, 1:2], s2p[:], axis=mybir.AxisListType.X)
        s1t = stats.tile([C, 1], f32)
        nc.vector.reduce_sum(s1t[:], s1p[:], axis=mybir.AxisListType.X)
        tmpc = stats.tile([C, 1], f32)
        nc.vector.tensor_scalar_mul(tmpc[:], beff[:], float(HW))
        nc.vector.tensor_sub(st2[:, 0:1], s1t[:], tmpc[:])

        with tc.tile_pool(name="pa", bufs=2, space="PSUM") as pa:
            gs_ps = pa.tile([GROUPS, 2], f32, tag="qk")
            nc.tensor.matmul(gs_ps[:], ig[:], st2[:], start=True, stop=True)
            gstats = stats.tile([GROUPS, 2], f32)
            nc.vector.tensor_copy(gstats[:], gs_ps[:])
            inv_n = 1.0 / float(GSIZE * HW)
            gmean = stats.tile([GROUPS, 1], f32)
            nc.vector.tensor_scalar_mul(gmean[:], gstats[:, 0:1], inv_n)
            gm2 = stats.tile([GROUPS, 1], f32)
            nc.vector.tensor_scalar_mul(gm2[:], gstats[:, 1:2], inv_n)
            gmsq = stats.tile([GROUPS, 1], f32)
            nc.vector.tensor_mul(gmsq[:], gmean[:], gmean[:])
            gvar = stats.tile([GROUPS, 1], f32)
            nc.vector.tensor_sub(gvar[:], gm2[:], gmsq[:])
            gsd = stats.tile([GROUPS, 1], f32)
            nc.scalar.activation(gsd[:], gvar[:], AF.Sqrt, bias=eps_t[:])
            gmr = stats.tile([GROUPS, 2], f32)
            nc.vector.reciprocal(gmr[:, 1:2], gsd[:])
            nc.vector.tensor_copy(gmr[:, 0:1], gmean[:])

            bc_ps = pa.tile([C, 2], f32, tag="qk")
            nc.tensor.matmul(bc_ps[:], igt[:], gmr[:], start=True, stop=True)
            a_c = stats.tile([C, 1], f32)
            b_c = stats.tile([C, 1], f32)
            tmc = stats.tile([C, 1], f32)
            nc.vector.tensor_scalar_mul(a_c[:], gamma[:], bc_ps[:, 1:2])
            nc.vector.tensor_scalar_mul(tmc[:], a_c[:], bc_ps[:, 0:1])
            nc.vector.tensor_sub(b_c[:], beta[:], tmc[:])

            # hn = a_c * x + b_c   (fp32r)
            hn = big.tile([C, HW], f32r, tag="hn")
            nc.scalar.activation(hn[:], x_sb[:], AF.Identity, bias=b_c[:], scale=a_c[:])

            # ---------------- QKV ----------------
            q_r = big.tile([C, HW], f32r, tag="q")
            k_r = big.tile([C, HW], f32r, tag="k")
            vt = big.tile([C, NJ, C], f32r, tag="vt")
            for h in range(NHALF):
                qp = pa.tile([C, HWID], f32, tag="qk")
                for kk in range(4):
                    sl = slice(kk * IBLK, (kk + 1) * IBLK)
                    nc.tensor.matmul(
                        qp[:, sl], wq[:], hn[:, h * HWID + kk * IBLK:h * HWID + (kk + 1) * IBLK],
                        start=True, stop=True,
                    )
                nc.scalar.activation(
                    q_r[:, h * HWID:(h + 1) * HWID], qp[:], AF.Identity, bias=bq[:]
                )
            for h in range(NHALF):
                kp = pa.tile([C, HWID], f32, tag="qk")
                for kk in range(4):
                    sl = slice(kk * IBLK, (kk + 1) * IBLK)
                    nc.tensor.matmul(
                        kp[:, sl], wk[:], hn[:, h * HWID + kk * IBLK:h * HWID + (kk + 1) * IBLK],
                        start=True, stop=True,
                    )
                # K copy on DVE (keeps ACT free); b_k added via tensor_scalar
                nc.vector.tensor_scalar(
                    k_r[:, h * HWID:(h + 1) * HWID], kp[:], bk[:], None,
                    mybir.AluOpType.add,
                )
            for r in range(4):
                vp = pa.tile([C, 8, 2 * C], f32, tag="qk")
                for t in range(8):
                    nt = r * 8 + t
                    nc.tensor.matmul(
                        vp[:, t, :], hn[:, nt * 128:(nt + 1) * 128], wv2[:],
                        start=True, stop=True,
                    )
                nc.vector.tensor_copy(vt[:, r * 8:(r + 1) * 8, :], vp[:, :, 0:C])

        # ---------------- main attention loop ----------------
        # PSUM: two independently-released S tiles (2 banks each) + one O
        # accumulator (4 banks). exp is split into two [C, 1024] halves so
        # the S banks free as soon as their half is consumed -> the next
        # iteration's S-matmuls overlap the current exp (keeps PE dense and
        # the HAM clock warm).
        acc = big.tile([C, HW], f32, tag="x")  # reuses x slot (x is dead)
        o_sb = big.tile([C, HW], f32, tag="o")
        HQ = HWID // 2  # 1024
        with tc.tile_pool(name="ps_s", bufs=1, space="PSUM") as ps_s, \
             tc.tile_pool(name="ps_o", bufs=1, space="PSUM") as ps_o:
            for h in range(NHALF):
                hsl = slice(h * HWID, (h + 1) * HWID)
                op = ps_o.tile([C, HWID], f32, tag="o_ps")
                for j in range(NJ):
                    pt = ptpool.tile([C, HWID], f32r)
                    for half in range(2):
                        sp = ps_s.tile([C, HQ], f32, tag=f"s{half}")
                        for kk in range(2):
                            qoff = h * HWID + half * HQ + kk * IBLK
                            nc.tensor.matmul(
                                sp[:, kk * IBLK:(kk + 1) * IBLK],
                                k_r[:, j * 128:(j + 1) * 128],
                                q_r[:, qoff:qoff + IBLK],
                                start=True, stop=True,
                            )
                        nc.scalar.activation(
                            pt[:, half * HQ:(half + 1) * HQ], sp[:], AF.Exp
                        )
                    if j == 0:
                        nc.vector.tensor_copy(acc[:, hsl], pt[:].bitcast(f32))
                    else:
                        nc.vector.tensor_add(acc[:, hsl], acc[:, hsl], pt[:].bitcast(f32))
                    for kk in range(4):
                        sl = slice(kk * IBLK, (kk + 1) * IBLK)
                        nc.tensor.matmul(
                            op[:, sl], vt[:, j, :], pt[:, sl],
                            start=(j == 0), stop=(j == NJ - 1),
                        )
                nc.scalar.activation(o_sb[:, hsl], op[:], AF.Copy)

        # ---------------- softmax denominators + projection ----------------
        recip = row.tile([1, HW], f32r)
        with tc.tile_pool(name="prs", bufs=2, space="PSUM") as prs:
            for ib in range(NIB):
                sl = slice(ib * IBLK, (ib + 1) * IBLK)
                rp = prs.tile([1, IBLK], f32)
                nc.tensor.matmul(rp[:], ones_c[:], acc[:, sl], start=True, stop=True)
                with nc.allow_low_precision(reason="fp32r rounding of softmax recip is benign"):
                    nc.vector.reciprocal(recip[:, sl], rp[:])

        o_nrm = big.tile([C, HW], f32r, tag="hn")  # reuses hn slot
        with tc.tile_pool(name="pbc", bufs=2, space="PSUM") as pbc:
            for ib in range(NIB):
                sl = slice(ib * IBLK, (ib + 1) * IBLK)
                bp = pbc.tile([C, IBLK], f32)
                nc.tensor.matmul(bp[:], ones_r[:], recip[:, sl], start=True, stop=True)
                nc.vector.tensor_mul(o_nrm[:, sl], o_sb[:, sl], bp[:])

        out_sb = big.tile([C, HW], f32, tag="scratch")  # reuses xsq slot
        with tc.tile_pool(name="ppj", bufs=2, space="PSUM") as ppj:
            for ib in range(NIB):
                sl = slice(ib * IBLK, (ib + 1) * IBLK)
                pp = ppj.tile([C, IBLK], f32)
                nc.tensor.matmul(pp[:], wp[:], o_nrm[:, sl], start=True, stop=True)
                nc.vector.tensor_add(out_sb[:, sl], xres[:, sl], pp[:])
                nc.sync.dma_start(out_dram[:, sl], out_sb[:, sl])

    nc.compile()
    return nc


def _get_nc():
    if "nc" not in _CACHE:
        _CACHE["nc"] = _build()
    return _CACHE["nc"]


def _prep_inputs(x, gamma, beta, w_qkv, b_qkv, w_proj, b_proj):
    x = np.ascontiguousarray(x, dtype=np.float32)
    w_qkv = np.asarray(w_qkv, dtype=np.float32)
    b_qkv = np.asarray(b_qkv, dtype=np.float32)
    w_proj = np.asarray(w_proj, dtype=np.float32)
    b_proj = np.asarray(b_proj, dtype=np.float32)

    wq = w_qkv[0:C, :]
    wk = w_qkv[C:2 * C, :]
    wv = w_qkv[2 * C:3 * C, :]
    bqv = b_qkv[0:C]
    bkv = b_qkv[C:2 * C]
    bvv = b_qkv[2 * C:3 * C]

    wqT = np.ascontiguousarray((wq * SCALE).T)
    wkT = np.ascontiguousarray(wk.T)
    wvT2 = np.concatenate(
        [np.ascontiguousarray(wv.T), np.zeros((C, C), np.float32)], axis=1
    )
    wpT = np.ascontiguousarray(w_proj.T)
    beff = (b_proj + w_proj @ bvv).astype(np.float32)

    ig = np.zeros((C, GROUPS), np.float32)
    ig[np.arange(C), np.arange(C) // GSIZE] = 1.0
    igt = np.ascontiguousarray(ig.T)

    common = {
        "gamma": np.asarray(gamma, np.float32).reshape(C, 1),
        "beta": np.asarray(beta, np.float32).reshape(C, 1),
        "bq": (bqv * SCALE).reshape(C, 1),
        "bk": bkv.reshape(C, 1),
        "beff": beff.reshape(C, 1),
        "wqT": wqT,
        "wkT": wkT,
        "wvT2": np.ascontiguousarray(wvT2),
        "wpT": wpT,
        "ig": ig,
        "igt": igt,
    }
    in_maps = []
    for b in range(B):
        m = dict(common)
        m["x"] = np.ascontiguousarray(x[b].reshape(C, HW))
        in_maps.append(m)
    return in_maps


def kernel(x, gamma, beta, w_qkv, b_qkv, w_proj, b_proj):
    from concourse.bass_utils import run_bass_kernel_spmd

    nc = _get_nc()
    in_maps = _prep_inputs(x, gamma, beta, w_qkv, b_qkv, w_proj, b_proj)
    res = run_bass_kernel_spmd(nc, in_maps, list(range(B)))
    out = np.stack([res.results[b]["out"] for b in range(B)], axis=0)
    return out.reshape(B, C, H, W).astype(np.float32)


# revision 18
# speedup vs baseline: 2.1715x; 1.2582x over previous
"""Trainium2 Bass kernel for nn_AttentionBlock (GroupNorm + 1x1-conv QKV +
full self-attention over N=HW=4096 + output projection + residual).

Distribution: data-parallel over batch B=8, one batch element per NeuronCore.

Per-core layout / algorithm (C=128 channels on SBUF partitions, N=4096 free):
  1. GroupNorm stats via two ACT passes (Square + Identity, both with
     accum_out row-sums), cross-partition group combine via tiny indicator
     matmuls on the PE.
  2. hn = a_c * x + b_c  (one ACT pass, output rounded to fp32r).
  3. Q, K in natural [c, n] layout (lhsT = host-pretransposed weights);
     V^T in [n, c] tile-major layout (lhsT = hn tiles). All fp32r.
  4. Main loop (2 halves x 32 j-tiles): S^T tile = K_j^T Q  (PE, fp32r),
     P^T = exp(S^T) (ACT, -> fp32r), acc += P^T (DVE, fp32 denominator
     partials), O += V^T_j^T P^T (PE accumulate in PSUM).
     No max-subtraction: logits are ~N(0,1) so exp is safe in fp32.
  5. rowsum = ones^T acc (PE), recip = 1/rowsum, broadcast via K=1 outer
     product matmul, O_norm = O * recip (DVE), proj = w_proj^T O_norm (PE),
     out = (x + b_eff) + proj, streamed to DRAM per 512-block.

Bias algebra: b_q, b_k folded into the Q/K PSUM->SBUF copies (ACT bias);
b_v folded into b_eff = b_proj + w_proj @ b_v (host precompute, exact).
The attention scale C^-0.5 is folded into w_q/b_q on the host (exact
linear-map reparameterization).
"""

import numpy as np

B, C, H, W = 8, 128, 64, 64
HW = H * W                      # 4096
GROUPS = 8
GSIZE = C // GROUPS             # 16
EPS = 1e-5
NJ = HW // 128                  # 32 j-tiles
IBLK = 512
NIB = HW // IBLK                # 8 i-blocks
NHALF = 2
HWID = HW // NHALF              # 2048
SCALE = float(C) ** -0.5

_CACHE = {}


def _build():
    from contextlib import ExitStack

    import concourse.bacc as bacc
    import concourse.tile as tile
    from concourse import mybir

    f32 = mybir.dt.float32
    f32r = mybir.dt.float32r
    AF = mybir.ActivationFunctionType

    nc = bacc.Bacc("TRN2", target_bir_lowering=False, debug=False)

    x_in = nc.dram_tensor("x", [C, HW], f32, kind="ExternalInput")
    gamma_in = nc.dram_tensor("gamma", [C, 1], f32, kind="ExternalInput")
    beta_in = nc.dram_tensor("beta", [C, 1], f32, kind="ExternalInput")
    bq_in = nc.dram_tensor("bq", [C, 1], f32, kind="ExternalInput")
    bk_in = nc.dram_tensor("bk", [C, 1], f32, kind="ExternalInput")
    beff_in = nc.dram_tensor("beff", [C, 1], f32, kind="ExternalInput")
    wq_in = nc.dram_tensor("wqT", [C, C], f32, kind="ExternalInput")
    wk_in = nc.dram_tensor("wkT", [C, C], f32, kind="ExternalInput")
    wv_in = nc.dram_tensor("wvT2", [C, 2 * C], f32, kind="ExternalInput")
    wp_in = nc.dram_tensor("wpT", [C, C], f32, kind="ExternalInput")
    ig_in = nc.dram_tensor("ig", [C, GROUPS], f32, kind="ExternalInput")
    igt_in = nc.dram_tensor("igt", [GROUPS, C], f32, kind="ExternalInput")
    out_dram = nc.dram_tensor("out", [C, HW], f32, kind="ExternalOutput")

    with tile.TileContext(nc) as tc, ExitStack() as ctx:
        const = ctx.enter_context(tc.tile_pool(name="const", bufs=1))
        big = ctx.enter_context(tc.tile_pool(name="big", bufs=1))
        stats = ctx.enter_context(tc.tile_pool(name="stats", bufs=1))
        ptpool = ctx.enter_context(tc.tile_pool(name="pt", bufs=3))
        row = ctx.enter_context(tc.tile_pool(name="row", bufs=1))

        # ---------------- constants ----------------
        def cload(t_in, shape, tag):
            t = const.tile(shape, f32, tag=tag)
            nc.sync.dma_start(t[:], t_in[:])
            return t

        gamma = cload(gamma_in, [C, 1], "c_gamma")
        beta = cload(beta_in, [C, 1], "c_beta")
        bq = cload(bq_in, [C, 1], "c_bq")
        bk = cload(bk_in, [C, 1], "c_bk")
        beff = cload(beff_in, [C, 1], "c_beff")
        ig = cload(ig_in, [C, GROUPS], "c_ig")
        igt = cload(igt_in, [GROUPS, C], "c_igt")
        wq_f = cload(wq_in, [C, C], "c_wq_f")
        wk_f = cload(wk_in, [C, C], "c_wk_f")
        wv_f = cload(wv_in, [C, 2 * C], "c_wv_f")
        wp_f = cload(wp_in, [C, C], "c_wp_f")

        wq = const.tile([C, C], f32r)
        nc.vector.tensor_copy(wq[:], wq_f[:])
        wk = const.tile([C, C], f32r)
        nc.vector.tensor_copy(wk[:], wk_f[:])
        wv2 = const.tile([C, 2 * C], f32r)
        nc.vector.tensor_copy(wv2[:], wv_f[:])
        wp = const.tile([C, C], f32r)
        nc.vector.tensor_copy(wp[:], wp_f[:])

        ones_c = const.tile([C, 1], f32)
        nc.vector.memset(ones_c[:], 1.0)
        ones_r = const.tile([1, C], f32r)
        nc.vector.memset(ones_r[:].bitcast(f32), 1.0)
        eps_t = const.tile([GROUPS, 1], f32)
        nc.vector.memset(eps_t[:], EPS)

        # ---------------- load x (chunked; stats split across DVE and ACT
        # so they overlap the DMA and each other) ----------------
        NCH = 4
        CHW = HW // NCH
        x_sb = big.tile([C, HW], f32, tag="x")
        for ch in range(NCH):
            sl = slice(ch * CHW, (ch + 1) * CHW)
            nc.sync.dma_start(x_sb[:, sl], x_in[:, sl])

        st2 = stats.tile([C, 2], f32)
        s2p = stats.tile([C, NCH], f32)
        s1p = stats.tile([C, NCH], f32)
        adum = stats.tile([C, CHW], f32)
        for ch in range(NCH):  # x^2 sums on ACT, x sums on DVE
            sl = slice(ch * CHW, (ch + 1) * CHW)
            nc.scalar.activation(
                adum[:], x_sb[:, sl], AF.Square, accum_out=s2p[:, ch:ch + 1]
            )
            nc.vector.reduce_sum(
                s1p[:, ch:ch + 1], x_sb[:, sl], axis=mybir.AxisListType.X
            )
        nc.vector.reduce_sum(st2[:, 1:2], s2p[:], axis=mybir.AxisListType.X)
        nc.vector.reduce_sum(st2[:, 0:1], s1p[:], axis=mybir.AxisListType.X)

        with tc.tile_pool(name="pa", bufs=2, space="PSUM") as pa:
            gs_ps = pa.tile([GROUPS, 2], f32, tag="qk")
            nc.tensor.matmul(gs_ps[:], ig[:], st2[:], start=True, stop=True)
            gstats = stats.tile([GROUPS, 2], f32)
            nc.vector.tensor_copy(gstats[:], gs_ps[:])
            inv_n = 1.0 / float(GSIZE * HW)
            gmean = stats.tile([GROUPS, 1], f32)
            nc.vector.tensor_scalar_mul(gmean[:], gstats[:, 0:1], inv_n)
            gm2 = stats.tile([GROUPS, 1], f32)
            nc.vector.tensor_scalar_mul(gm2[:], gstats[:, 1:2], inv_n)
            gmsq = stats.tile([GROUPS, 1], f32)
            nc.vector.tensor_mul(gmsq[:], gmean[:], gmean[:])
            gvar = stats.tile([GROUPS, 1], f32)
            nc.vector.tensor_sub(gvar[:], gm2[:], gmsq[:])
            gsd = stats.tile([GROUPS, 1], f32)
            nc.scalar.activation(gsd[:], gvar[:], AF.Sqrt, bias=eps_t[:])
            gmr = stats.tile([GROUPS, 2], f32)
            nc.vector.reciprocal(gmr[:, 1:2], gsd[:])
            nc.vector.tensor_copy(gmr[:, 0:1], gmean[:])

            bc_ps = pa.tile([C, 2], f32, tag="qk")
            nc.tensor.matmul(bc_ps[:], igt[:], gmr[:], start=True, stop=True)
            a_c = stats.tile([C, 1], f32)
            b_c = stats.tile([C, 1], f32)
            tmc = stats.tile([C, 1], f32)
            nc.vector.tensor_scalar_mul(a_c[:], gamma[:], bc_ps[:, 1:2])
            nc.vector.tensor_scalar_mul(tmc[:], a_c[:], bc_ps[:, 0:1])
            nc.vector.tensor_sub(b_c[:], beta[:], tmc[:])

            # hn = a_c * x + b_c   (fp32r), per half so QKV can start early
            hn = big.tile([C, HW], f32r, tag="hn")
            q_r = big.tile([C, HW], f32r, tag="q")
            k_r = big.tile([C, HW], f32r, tag="k")
            vt = big.tile([C, NJ, C], f32r, tag="vt")
            xres = big.tile([C, HW], f32, tag="xres")

            def emit_qkv_half(h):
                hs = slice(h * HWID, (h + 1) * HWID)
                nc.scalar.activation(
                    hn[:, hs], x_sb[:, hs], AF.Identity, bias=b_c[:], scale=a_c[:]
                )
                kp = pa.tile([C, HWID], f32, tag="qk")
                for kk in range(4):
                    sl = slice(kk * IBLK, (kk + 1) * IBLK)
                    off = h * HWID + kk * IBLK
                    nc.tensor.matmul(
                        kp[:, sl], wk[:], hn[:, off:off + IBLK], start=True, stop=True
                    )
                # K copy on DVE (keeps ACT free); b_k added via tensor_scalar
                nc.vector.tensor_scalar(
                    k_r[:, hs], kp[:], bk[:], None, mybir.AluOpType.add
                )
                for r in range(2):
                    vp = pa.tile([C, 8, 2 * C], f32, tag="qk")
                    for t in range(8):
                        nt = h * 16 + r * 8 + t
                        nc.tensor.matmul(
                            vp[:, t, :], hn[:, nt * 128:(nt + 1) * 128], wv2[:],
                            start=True, stop=True,
                        )
                    nc.vector.tensor_copy(
                        vt[:, h * 16 + r * 8:h * 16 + (r + 1) * 8, :], vp[:, :, 0:C]
                    )
                qp = pa.tile([C, HWID], f32, tag="qk")
                for kk in range(4):
                    sl = slice(kk * IBLK, (kk + 1) * IBLK)
                    off = h * HWID + kk * IBLK
                    nc.tensor.matmul(
                        qp[:, sl], wq[:], hn[:, off:off + IBLK], start=True, stop=True
                    )
                nc.scalar.activation(q_r[:, hs], qp[:], AF.Identity, bias=bq[:])

            emit_qkv_half(0)
            emit_qkv_half(1)

        # ---------------- main attention loop ----------------
        # PSUM: two independently-released S tiles (2 banks each) + one O
        # accumulator (4 banks). exp is split into two [C, 1024] halves so
        # the S banks free as soon as their half is consumed -> the next
        # iteration's S-matmuls overlap the current exp (keeps PE dense and
        # the HAM clock warm).
        acc = big.tile([C, HW], f32, tag="acc")
        o_sb = big.tile([C, HW], f32, tag="o")
        HQ = HWID // 2  # 1024
        with tc.tile_pool(name="ps_s", bufs=1, space="PSUM") as ps_s, \
             tc.tile_pool(name="ps_o", bufs=1, space="PSUM") as ps_o:
            for h in range(NHALF):
                hsl = slice(h * HWID, (h + 1) * HWID)
                op = ps_o.tile([C, HWID], f32, tag="o_ps")
                for j in range(NJ):
                    pt = ptpool.tile([C, HWID], f32r)
                    for half in range(2):
                        sp = ps_s.tile([C, HQ], f32, tag=f"s{half}")
                        for kk in range(2):
                            qoff = h * HWID + half * HQ + kk * IBLK
                            nc.tensor.matmul(
                                sp[:, kk * IBLK:(kk + 1) * IBLK],
                                k_r[:, j * 128:(j + 1) * 128],
                                q_r[:, qoff:qoff + IBLK],
                                start=True, stop=True,
                            )
                        nc.scalar.activation(
                            pt[:, half * HQ:(half + 1) * HQ], sp[:], AF.Exp
                        )
                    if j == 0:
                        nc.vector.tensor_copy(acc[:, hsl], pt[:].bitcast(f32))
                    else:
                        nc.vector.tensor_add(acc[:, hsl], acc[:, hsl], pt[:].bitcast(f32))
                    for kk in range(4):
                        sl = slice(kk * IBLK, (kk + 1) * IBLK)
                        nc.tensor.matmul(
                            op[:, sl], vt[:, j, :], pt[:, sl],
                            start=(j == 0), stop=(j == NJ - 1),
                        )
                nc.scalar.activation(o_sb[:, hsl], op[:], AF.Copy)

        # residual base (overlaps the denominator chain below on ACT)
        for h in range(NHALF):
            hs = slice(h * HWID, (h + 1) * HWID)
            nc.scalar.activation(xres[:, hs], x_sb[:, hs], AF.Identity, bias=beff[:])

        # ---------------- softmax denominators + projection ----------------
        # DVE reciprocal is ~8 cyc/elem *per lane*; a [1, 4096] layout would
        # put all 4096 on one lane (~34us). Round-trip through a [128, 32]
        # layout via DMA so all lanes work (~0.3us).
        recip = row.tile([1, HW], f32r)
        rs128 = stats.tile([C, HW // C], f32)  # [128, 32]
        rc128 = stats.tile([C, HW // C], f32r)
        with tc.tile_pool(name="prs", bufs=2, space="PSUM") as prs, \
             tc.tile_pool(name="stage", bufs=2) as stg:
            for ib in range(NIB):
                sl = slice(ib * IBLK, (ib + 1) * IBLK)
                rp = prs.tile([1, IBLK], f32)
                nc.tensor.matmul(rp[:], ones_c[:], acc[:, sl], start=True, stop=True)
                st = stg.tile([1, IBLK], f32, tag="stage")
                nc.vector.tensor_copy(st[:], rp[:])
                # scatter [1, 512] -> 16 partitions x 32
                nc.sync.dma_start(rs128[ib * 16:(ib + 1) * 16, :], st[:])
            with nc.allow_low_precision(reason="fp32r rounding of softmax recip is benign"):
                nc.vector.reciprocal(rc128[:], rs128[:])
            nc.sync.dma_start(recip[:], rc128[:])

        o_nrm = big.tile([C, HW], f32r, tag="hn")  # reuses hn slot
        with tc.tile_pool(name="pbc", bufs=2, space="PSUM") as pbc:
            for ib in range(NIB):
                sl = slice(ib * IBLK, (ib + 1) * IBLK)
                bp = pbc.tile([C, IBLK], f32)
                nc.tensor.matmul(bp[:], ones_r[:], recip[:, sl], start=True, stop=True)
                nc.vector.tensor_mul(o_nrm[:, sl], o_sb[:, sl], bp[:])

        out_sb = big.tile([C, HW], f32, tag="scratch")  # reuses xsq slot
        with tc.tile_pool(name="ppj", bufs=2, space="PSUM") as ppj:
            for ib in range(NIB):
                sl = slice(ib * IBLK, (ib + 1) * IBLK)
                pp = ppj.tile([C, IBLK], f32)
                nc.tensor.matmul(pp[:], wp[:], o_nrm[:, sl], start=True, stop=True)
                nc.vector.tensor_add(out_sb[:, sl], xres[:, sl], pp[:])
                nc.sync.dma_start(out_dram[:, sl], out_sb[:, sl])

    nc.compile()
    return nc


def _get_nc():
    if "nc" not in _CACHE:
        _CACHE["nc"] = _build()
    return _CACHE["nc"]


def _prep_inputs(x, gamma, beta, w_qkv, b_qkv, w_proj, b_proj):
    x = np.ascontiguousarray(x, dtype=np.float32)
    w_qkv = np.asarray(w_qkv, dtype=np.float32)
    b_qkv = np.asarray(b_qkv, dtype=np.float32)
    w_proj = np.asarray(w_proj, dtype=np.float32)
    b_proj = np.asarray(b_proj, dtype=np.float32)

    wq = w_qkv[0:C, :]
    wk = w_qkv[C:2 * C, :]
    wv = w_qkv[2 * C:3 * C, :]
    bqv = b_qkv[0:C]
    bkv = b_qkv[C:2 * C]
    bvv = b_qkv[2 * C:3 * C]

    wqT = np.ascontiguousarray((wq * SCALE).T)
    wkT = np.ascontiguousarray(wk.T)
    wvT2 = np.concatenate(
        [np.ascontiguousarray(wv.T), np.zeros((C, C), np.float32)], axis=1
    )
    wpT = np.ascontiguousarray(w_proj.T)
    beff = (b_proj + w_proj @ bvv).astype(np.float32)

    ig = np.zeros((C, GROUPS), np.float32)
    ig[np.arange(C), np.arange(C) // GSIZE] = 1.0
    igt = np.ascontiguousarray(ig.T)

    common = {
        "gamma": np.asarray(gamma, np.float32).reshape(C, 1),
        "beta": np.asarray(beta, np.float32).reshape(C, 1),
        "bq": (bqv * SCALE).reshape(C, 1),
        "bk": bkv.reshape(C, 1),
        "beff": beff.reshape(C, 1),
        "wqT": wqT,
        "wkT": wkT,
        "wvT2": np.ascontiguousarray(wvT2),
        "wpT": wpT,
        "ig": ig,
        "igt": igt,
    }
    in_maps = []
    for b in range(B):
        m = dict(common)
        m["x"] = np.ascontiguousarray(x[b].reshape(C, HW))
        in_maps.append(m)
    return in_maps


def kernel(x, gamma, beta, w_qkv, b_qkv, w_proj, b_proj):
    from concourse.bass_utils import run_bass_kernel_spmd

    nc = _get_nc()
    in_maps = _prep_inputs(x, gamma, beta, w_qkv, b_qkv, w_proj, b_proj)
    res = run_bass_kernel_spmd(nc, in_maps, list(range(B)))
    out = np.stack([res.results[b]["out"] for b in range(B)], axis=0)
    return out.reshape(B, C, H, W).astype(np.float32)


# revision 20
# speedup vs baseline: 2.2234x; 1.0239x over previous
"""Trainium2 Bass kernel for nn_AttentionBlock (GroupNorm + 1x1-conv QKV +
full self-attention over N=HW=4096 + output projection + residual).

Distribution: data-parallel over batch B=8, one batch element per NeuronCore.

Per-core layout / algorithm (C=128 channels on SBUF partitions, N=4096 free):
  1. GroupNorm stats via two ACT passes (Square + Identity, both with
     accum_out row-sums), cross-partition group combine via tiny indicator
     matmuls on the PE.
  2. hn = a_c * x + b_c  (one ACT pass, output rounded to fp32r).
  3. Q, K in natural [c, n] layout (lhsT = host-pretransposed weights);
     V^T in [n, c] tile-major layout (lhsT = hn tiles). All fp32r.
  4. Main loop (2 halves x 32 j-tiles): S^T tile = K_j^T Q  (PE, fp32r),
     P^T = exp(S^T) (ACT, -> fp32r), acc += P^T (DVE, fp32 denominator
     partials), O += V^T_j^T P^T (PE accumulate in PSUM).
     No max-subtraction: logits are ~N(0,1) so exp is safe in fp32.
  5. rowsum = ones^T acc (PE), recip = 1/rowsum, broadcast via K=1 outer
     product matmul, O_norm = O * recip (DVE), proj = w_proj^T O_norm (PE),
     out = (x + b_eff) + proj, streamed to DRAM per 512-block.

Bias algebra: b_q, b_k folded into the Q/K PSUM->SBUF copies (ACT bias);
b_v folded into b_eff = b_proj + w_proj @ b_v (host precompute, exact).
The attention scale C^-0.5 is folded into w_q/b_q on the host (exact
linear-map reparameterization).
"""

import numpy as np

B, C, H, W = 8, 128, 64, 64
HW = H * W                      # 4096
GROUPS = 8
GSIZE = C // GROUPS             # 16
EPS = 1e-5
NJ = HW // 128                  # 32 j-tiles
IBLK = 512
NIB = HW // IBLK                # 8 i-blocks
NHALF = 2
HWID = HW // NHALF              # 2048
SCALE = float(C) ** -0.5

_CACHE = {}


def _build():
    from contextlib import ExitStack

    import concourse.bacc as bacc
    import concourse.tile as tile
    from concourse import mybir

    f32 = mybir.dt.float32
    f32r = mybir.dt.float32r
    AF = mybir.ActivationFunctionType

    nc = bacc.Bacc("TRN2", target_bir_lowering=False, debug=False)

    x_in = nc.dram_tensor("x", [C, HW], f32, kind="ExternalInput")
    gamma_in = nc.dram_tensor("gamma", [C, 1], f32, kind="ExternalInput")
    beta_in = nc.dram_tensor("beta", [C, 1], f32, kind="ExternalInput")
    bq_in = nc.dram_tensor("bq", [C, 1], f32, kind="ExternalInput")
    bk_in = nc.dram_tensor("bk", [C, 1], f32, kind="ExternalInput")
    beff_in = nc.dram_tensor("beff", [C, 1], f32, kind="ExternalInput")
    wq_in = nc.dram_tensor("wqT", [C, C], f32, kind="ExternalInput")
    wk_in = nc.dram_tensor("wkT", [C, C], f32, kind="ExternalInput")
    wv_in = nc.dram_tensor("wvT2", [C, 2 * C], f32, kind="ExternalInput")
    wp_in = nc.dram_tensor("wpT", [C, C], f32, kind="ExternalInput")
    ig_in = nc.dram_tensor("ig", [C, GROUPS], f32, kind="ExternalInput")
    igt_in = nc.dram_tensor("igt", [GROUPS, C], f32, kind="ExternalInput")
    out_dram = nc.dram_tensor("out", [C, HW], f32, kind="ExternalOutput")

    with tile.TileContext(nc) as tc, ExitStack() as ctx:
        const = ctx.enter_context(tc.tile_pool(name="const", bufs=1))
        big = ctx.enter_context(tc.tile_pool(name="big", bufs=1))
        stats = ctx.enter_context(tc.tile_pool(name="stats", bufs=1))
        ptpool = ctx.enter_context(tc.tile_pool(name="pt", bufs=3))
        row = ctx.enter_context(tc.tile_pool(name="row", bufs=1))

        # ---------------- load x first (sync queue), consts on the ACT
        # HWDGE queue so they don't serialize behind/ahead of x ----------------
        NCH = 4
        CHW = HW // NCH
        x_sb = big.tile([C, HW], f32, tag="x")
        for ch in range(NCH):
            sl = slice(ch * CHW, (ch + 1) * CHW)
            nc.sync.dma_start(x_sb[:, sl], x_in[:, sl])

        def cload(t_in, shape, tag):
            t = const.tile(shape, f32, tag=tag)
            nc.scalar.dma_start(t[:], t_in[:])
            return t

        gamma = cload(gamma_in, [C, 1], "c_gamma")
        beta = cload(beta_in, [C, 1], "c_beta")
        bq = cload(bq_in, [C, 1], "c_bq")
        bk = cload(bk_in, [C, 1], "c_bk")
        beff = cload(beff_in, [C, 1], "c_beff")
        ig = cload(ig_in, [C, GROUPS], "c_ig")
        igt = cload(igt_in, [GROUPS, C], "c_igt")
        wq_f = cload(wq_in, [C, C], "c_wq_f")
        wk_f = cload(wk_in, [C, C], "c_wk_f")
        wv_f = cload(wv_in, [C, 2 * C], "c_wv_f")
        wp_f = cload(wp_in, [C, C], "c_wp_f")

        wq = const.tile([C, C], f32r)
        nc.vector.tensor_copy(wq[:], wq_f[:])
        wk = const.tile([C, C], f32r)
        nc.vector.tensor_copy(wk[:], wk_f[:])
        wv2 = const.tile([C, 2 * C], f32r)
        nc.vector.tensor_copy(wv2[:], wv_f[:])
        wp = const.tile([C, C], f32r)
        nc.vector.tensor_copy(wp[:], wp_f[:])

        ones_c = const.tile([C, 1], f32)
        nc.vector.memset(ones_c[:], 1.0)
        ones_r = const.tile([1, C], f32r)
        nc.vector.memset(ones_r[:].bitcast(f32), 1.0)
        eps_t = const.tile([GROUPS, 1], f32)
        nc.vector.memset(eps_t[:], EPS)

        # ---------------- groupnorm stats (split across DVE and ACT) ----
        st2 = stats.tile([C, 2], f32)
        s2p = stats.tile([C, NCH], f32)
        s1p = stats.tile([C, NCH], f32)
        adum = stats.tile([C, CHW], f32)
        for ch in range(NCH):  # x^2 sums on ACT, x sums on DVE
            sl = slice(ch * CHW, (ch + 1) * CHW)
            nc.scalar.activation(
                adum[:], x_sb[:, sl], AF.Square, accum_out=s2p[:, ch:ch + 1]
            )
            nc.vector.reduce_sum(
                s1p[:, ch:ch + 1], x_sb[:, sl], axis=mybir.AxisListType.X
            )
        nc.vector.reduce_sum(st2[:, 1:2], s2p[:], axis=mybir.AxisListType.X)
        nc.vector.reduce_sum(st2[:, 0:1], s1p[:], axis=mybir.AxisListType.X)

        with tc.tile_pool(name="pa", bufs=2, space="PSUM") as pa:
            gs_ps = pa.tile([GROUPS, 2], f32, tag="qk")
            nc.tensor.matmul(gs_ps[:], ig[:], st2[:], start=True, stop=True)
            gstats = stats.tile([GROUPS, 2], f32)
            nc.vector.tensor_copy(gstats[:], gs_ps[:])
            inv_n = 1.0 / float(GSIZE * HW)
            gmean = stats.tile([GROUPS, 1], f32)
            nc.vector.tensor_scalar_mul(gmean[:], gstats[:, 0:1], inv_n)
            gm2 = stats.tile([GROUPS, 1], f32)
            nc.vector.tensor_scalar_mul(gm2[:], gstats[:, 1:2], inv_n)
            gmsq = stats.tile([GROUPS, 1], f32)
            nc.vector.tensor_mul(gmsq[:], gmean[:], gmean[:])
            gvar = stats.tile([GROUPS, 1], f32)
            nc.vector.tensor_sub(gvar[:], gm2[:], gmsq[:])
            gsd = stats.tile([GROUPS, 1], f32)
            nc.scalar.activation(gsd[:], gvar[:], AF.Sqrt, bias=eps_t[:])
            gmr = stats.tile([GROUPS, 2], f32)
            nc.vector.reciprocal(gmr[:, 1:2], gsd[:])
            nc.vector.tensor_copy(gmr[:, 0:1], gmean[:])

            bc_ps = pa.tile([C, 2], f32, tag="qk")
            nc.tensor.matmul(bc_ps[:], igt[:], gmr[:], start=True, stop=True)
            a_c = stats.tile([C, 1], f32)
            b_c = stats.tile([C, 1], f32)
            tmc = stats.tile([C, 1], f32)
            nc.vector.tensor_scalar_mul(a_c[:], gamma[:], bc_ps[:, 1:2])
            nc.vector.tensor_scalar_mul(tmc[:], a_c[:], bc_ps[:, 0:1])
            nc.vector.tensor_sub(b_c[:], beta[:], tmc[:])

            # hn = a_c * x + b_c   (fp32r), per half so QKV can start early
            hn = big.tile([C, HW], f32r, tag="hn")
            q_r = big.tile([C, HW], f32r, tag="q")
            k_r = big.tile([C, HW], f32r, tag="k")
            vt = big.tile([C, NJ, C], f32r, tag="vt")

            def emit_qkv_half(h):
                hs = slice(h * HWID, (h + 1) * HWID)
                nc.scalar.activation(
                    hn[:, hs], x_sb[:, hs], AF.Identity, bias=b_c[:], scale=a_c[:]
                )
                kp = pa.tile([C, HWID], f32, tag="qk")
                for kk in range(4):
                    sl = slice(kk * IBLK, (kk + 1) * IBLK)
                    off = h * HWID + kk * IBLK
                    nc.tensor.matmul(
                        kp[:, sl], wk[:], hn[:, off:off + IBLK], start=True, stop=True
                    )
                # K copy on DVE (keeps ACT free); b_k added via tensor_scalar
                nc.vector.tensor_scalar(
                    k_r[:, hs], kp[:], bk[:], None, mybir.AluOpType.add
                )
                for r in range(2):
                    vp = pa.tile([C, 8, 2 * C], f32, tag="qk")
                    for t in range(8):
                        nt = h * 16 + r * 8 + t
                        nc.tensor.matmul(
                            vp[:, t, :], hn[:, nt * 128:(nt + 1) * 128], wv2[:],
                            start=True, stop=True,
                        )
                    nc.vector.tensor_copy(
                        vt[:, h * 16 + r * 8:h * 16 + (r + 1) * 8, :], vp[:, :, 0:C]
                    )
                qp = pa.tile([C, HWID], f32, tag="qk")
                for kk in range(4):
                    sl = slice(kk * IBLK, (kk + 1) * IBLK)
                    off = h * HWID + kk * IBLK
                    nc.tensor.matmul(
                        qp[:, sl], wq[:], hn[:, off:off + IBLK], start=True, stop=True
                    )
                nc.scalar.activation(q_r[:, hs], qp[:], AF.Identity, bias=bq[:])

            emit_qkv_half(0)
            emit_qkv_half(1)

        # ---------------- main attention loop ----------------
        # PSUM: two independently-released S tiles (2 banks each) + one O
        # accumulator (4 banks). exp is split into two [C, 1024] halves so
        # the S banks free as soon as their half is consumed -> the next
        # iteration's S-matmuls overlap the current exp (keeps PE dense and
        # the HAM clock warm).
        acc = big.tile([C, HW], f32, tag="acc")
        o_sb = big.tile([C, HW], f32, tag="o")
        HQ = HWID // 2  # 1024
        with tc.tile_pool(name="ps_s", bufs=1, space="PSUM") as ps_s, \
             tc.tile_pool(name="ps_o", bufs=1, space="PSUM") as ps_o:
            for h in range(NHALF):
                hsl = slice(h * HWID, (h + 1) * HWID)
                op = ps_o.tile([C, HWID], f32, tag="o_ps")
                for j in range(NJ):
                    pt = ptpool.tile([C, HWID], f32r)
                    for half in range(2):
                        sp = ps_s.tile([C, HQ], f32, tag=f"s{half}")
                        for kk in range(2):
                            qoff = h * HWID + half * HQ + kk * IBLK
                            nc.tensor.matmul(
                                sp[:, kk * IBLK:(kk + 1) * IBLK],
                                k_r[:, j * 128:(j + 1) * 128],
                                q_r[:, qoff:qoff + IBLK],
                                start=True, stop=True,
                            )
                        nc.scalar.activation(
                            pt[:, half * HQ:(half + 1) * HQ], sp[:], AF.Exp
                        )
                    if j == 0:
                        nc.vector.tensor_copy(acc[:, hsl], pt[:].bitcast(f32))
                    else:
                        nc.vector.tensor_add(acc[:, hsl], acc[:, hsl], pt[:].bitcast(f32))
                    for kk in range(4):
                        sl = slice(kk * IBLK, (kk + 1) * IBLK)
                        nc.tensor.matmul(
                            op[:, sl], vt[:, j, :], pt[:, sl],
                            start=(j == 0), stop=(j == NJ - 1),
                        )
                nc.scalar.activation(o_sb[:, hsl], op[:], AF.Copy)

        # ---------------- softmax denominators + projection ----------------
        # DVE reciprocal is ~8 cyc/elem *per lane*; a [1, 4096] layout would
        # put all 4096 on one lane (~34us). Round-trip through a [128, 32]
        # layout via DMA so all lanes work (~0.3us).
        recip = row.tile([1, HW], f32r)
        rs128 = stats.tile([C, HW // C], f32)  # [128, 32]
        rc128 = stats.tile([C, HW // C], f32r)
        with tc.tile_pool(name="prs", bufs=3, space="PSUM") as prs, \
             tc.tile_pool(name="stage", bufs=3) as stg:
            for ib in range(NIB):
                sl = slice(ib * IBLK, (ib + 1) * IBLK)
                rp = prs.tile([1, IBLK], f32)
                nc.tensor.matmul(rp[:], ones_c[:], acc[:, sl], start=True, stop=True)
                st = stg.tile([1, IBLK], f32, tag="stage")
                nc.vector.tensor_copy(st[:], rp[:])
                # scatter [1, 512] -> 16 partitions x 32
                nc.sync.dma_start(rs128[ib * 16:(ib + 1) * 16, :], st[:])
            with nc.allow_low_precision(reason="fp32r rounding of softmax recip is benign"):
                nc.vector.reciprocal(rc128[:], rs128[:])
            nc.sync.dma_start(recip[:], rc128[:])

        o_nrm = big.tile([C, HW], f32r, tag="hn")  # reuses hn slot
        with tc.tile_pool(name="pbc", bufs=2, space="PSUM") as pbc:
            for ib in range(NIB):
                sl = slice(ib * IBLK, (ib + 1) * IBLK)
                bp = pbc.tile([C, IBLK], f32)
                nc.tensor.matmul(bp[:], ones_r[:], recip[:, sl], start=True, stop=True)
                nc.vector.tensor_mul(o_nrm[:, sl], o_sb[:, sl], bp[:])

        out_sb = big.tile([C, HW], f32, tag="scratch")  # reuses xsq slot
        with tc.tile_pool(name="ppj", bufs=2, space="PSUM") as ppj:
            for ib in range(NIB):
                sl = slice(ib * IBLK, (ib + 1) * IBLK)
                pp = ppj.tile([C, IBLK], f32)
                nc.tensor.matmul(pp[:], wp[:], o_nrm[:, sl], start=True, stop=True)
                nc.vector.tensor_scalar(
                    out_sb[:, sl], pp[:], beff[:], None, mybir.AluOpType.add
                )
                nc.vector.tensor_add(out_sb[:, sl], out_sb[:, sl], x_sb[:, sl])
                nc.sync.dma_start(out_dram[:, sl], out_sb[:, sl])

    nc.compile()
    return nc


def _get_nc():
    if "nc" not in _CACHE:
        _CACHE["nc"] = _build()
    return _CACHE["nc"]


def _prep_inputs(x, gamma, beta, w_qkv, b_qkv, w_proj, b_proj):
    x = np.ascontiguousarray(x, dtype=np.float32)
    w_qkv = np.asarray(w_qkv, dtype=np.float32)
    b_qkv = np.asarray(b_qkv, dtype=np.float32)
    w_proj = np.asarray(w_proj, dtype=np.float32)
    b_proj = np.asarray(b_proj, dtype=np.float32)

    wq = w_qkv[0:C, :]
    wk = w_qkv[C:2 * C, :]
    wv = w_qkv[2 * C:3 * C, :]
    bqv = b_qkv[0:C]
    bkv = b_qkv[C:2 * C]
    bvv = b_qkv[2 * C:3 * C]

    wqT = np.ascontiguousarray((wq * SCALE).T)
    wkT = np.ascontiguousarray(wk.T)
    wvT2 = np.concatenate(
        [np.ascontiguousarray(wv.T), np.zeros((C, C), np.float32)], axis=1
    )
    wpT = np.ascontiguousarray(w_proj.T)
    beff = (b_proj + w_proj @ bvv).astype(np.float32)

    ig = np.zeros((C, GROUPS), np.float32)
    ig[np.arange(C), np.arange(C) // GSIZE] = 1.0
    igt = np.ascontiguousarray(ig.T)

    common = {
        "gamma": np.asarray(gamma, np.float32).reshape(C, 1),
        "beta": np.asarray(beta, np.float32).reshape(C, 1),
        "bq": (bqv * SCALE).reshape(C, 1),
        "bk": bkv.reshape(C, 1),
        "beff": beff.reshape(C, 1),
        "wqT": wqT,
        "wkT": wkT,
        "wvT2": np.ascontiguousarray(wvT2),
        "wpT": wpT,
        "ig": ig,
        "igt": igt,
    }
    in_maps = []
    for b in range(B):
        m = dict(common)
        m["x"] = np.ascontiguousarray(x[b].reshape(C, HW))
        in_maps.append(m)
    return in_maps


def kernel(x, gamma, beta, w_qkv, b_qkv, w_proj, b_proj):
    from concourse.bass_utils import run_bass_kernel_spmd

    nc = _get_nc()
    in_maps = _prep_inputs(x, gamma, beta, w_qkv, b_qkv, w_proj, b_proj)
    res = run_bass_kernel_spmd(nc, in_maps, list(range(B)))
    out = np.stack([res.results[b]["out"] for b in range(B)], axis=0)
    return out.reshape(B, C, H, W).astype(np.float32)


# revision 21
# speedup vs baseline: 2.2624x; 1.0176x over previous
"""Trainium2 Bass kernel for nn_AttentionBlock (GroupNorm + 1x1-conv QKV +
full self-attention over N=HW=4096 + output projection + residual).

Distribution: data-parallel over batch B=8, one batch element per NeuronCore.

Per-core layout / algorithm (C=128 channels on SBUF partitions, N=4096 free):
  1. GroupNorm stats via two ACT passes (Square + Identity, both with
     accum_out row-sums), cross-partition group combine via tiny indicator
     matmuls on the PE.
  2. hn = a_c * x + b_c  (one ACT pass, output rounded to fp32r).
  3. Q, K in natural [c, n] layout (lhsT = host-pretransposed weights);
     V^T in [n, c] tile-major layout (lhsT = hn tiles). All fp32r.
  4. Main loop (2 halves x 32 j-tiles): S^T tile = K_j^T Q  (PE, fp32r),
     P^T = exp(S^T) (ACT, -> fp32r), acc += P^T (DVE, fp32 denominator
     partials), O += V^T_j^T P^T (PE accumulate in PSUM).
     No max-subtraction: logits are ~N(0,1) so exp is safe in fp32.
  5. rowsum = ones^T acc (PE), recip = 1/rowsum, broadcast via K=1 outer
     product matmul, O_norm = O * recip (DVE), proj = w_proj^T O_norm (PE),
     out = (x + b_eff) + proj, streamed to DRAM per 512-block.

Bias algebra: b_q, b_k folded into the Q/K PSUM->SBUF copies (ACT bias);
b_v folded into b_eff = b_proj + w_proj @ b_v (host precompute, exact).
The attention scale C^-0.5 is folded into w_q/b_q on the host (exact
linear-map reparameterization).
"""

import numpy as np

B, C, H, W = 8, 128, 64, 64
HW = H * W                      # 4096
GROUPS = 8
GSIZE = C // GROUPS             # 16
EPS = 1e-5
NJ = HW // 128                  # 32 j-tiles
IBLK = 512
NIB = HW // IBLK                # 8 i-blocks
NHALF = 2
HWID = HW // NHALF              # 2048
SCALE = float(C) ** -0.5

_CACHE = {}


def _build():
    from contextlib import ExitStack

    import concourse.bacc as bacc
    import concourse.tile as tile
    from concourse import mybir

    f32 = mybir.dt.float32
    f32r = mybir.dt.float32r
    AF = mybir.ActivationFunctionType

    nc = bacc.Bacc("TRN2", target_bir_lowering=False, debug=False)

    x_in = nc.dram_tensor("x", [C, HW], f32, kind="ExternalInput")
    gamma_in = nc.dram_tensor("gamma", [C, 1], f32, kind="ExternalInput")
    beta_in = nc.dram_tensor("beta", [C, 1], f32, kind="ExternalInput")
    bq_in = nc.dram_tensor("bq", [C, 1], f32, kind="ExternalInput")
    bk_in = nc.dram_tensor("bk", [C, 1], f32, kind="ExternalInput")
    beff_in = nc.dram_tensor("beff", [C, 1], f32, kind="ExternalInput")
    wq_in = nc.dram_tensor("wqT", [C, C], f32, kind="ExternalInput")
    wk_in = nc.dram_tensor("wkT", [C, C], f32, kind="ExternalInput")
    wv_in = nc.dram_tensor("wvT2", [C, 2 * C], f32, kind="ExternalInput")
    wp_in = nc.dram_tensor("wpT", [C, C], f32, kind="ExternalInput")
    ig_in = nc.dram_tensor("ig", [C, GROUPS], f32, kind="ExternalInput")
    igt_in = nc.dram_tensor("igt", [GROUPS, C], f32, kind="ExternalInput")
    out_dram = nc.dram_tensor("out", [C, HW], f32, kind="ExternalOutput")

    with tile.TileContext(nc) as tc, ExitStack() as ctx:
        const = ctx.enter_context(tc.tile_pool(name="const", bufs=1))
        big = ctx.enter_context(tc.tile_pool(name="big", bufs=1))
        stats = ctx.enter_context(tc.tile_pool(name="stats", bufs=1))
        ptpool = ctx.enter_context(tc.tile_pool(name="pt", bufs=3))
        row = ctx.enter_context(tc.tile_pool(name="row", bufs=1))

        # ---------------- load x first (sync queue), consts on the ACT
        # HWDGE queue so they don't serialize behind/ahead of x ----------------
        NCH = 4
        CHW = HW // NCH
        x_sb = big.tile([C, HW], f32, tag="x")
        for ch in range(NCH):
            sl = slice(ch * CHW, (ch + 1) * CHW)
            nc.sync.dma_start(x_sb[:, sl], x_in[:, sl])

        def cload(t_in, shape, tag):
            t = const.tile(shape, f32, tag=tag)
            nc.sync.dma_start(t[:], t_in[:])
            return t

        gamma = cload(gamma_in, [C, 1], "c_gamma")
        beta = cload(beta_in, [C, 1], "c_beta")
        bq = cload(bq_in, [C, 1], "c_bq")
        bk = cload(bk_in, [C, 1], "c_bk")
        beff = cload(beff_in, [C, 1], "c_beff")
        ig = cload(ig_in, [C, GROUPS], "c_ig")
        igt = cload(igt_in, [GROUPS, C], "c_igt")
        wq_f = cload(wq_in, [C, C], "c_wq_f")
        wk_f = cload(wk_in, [C, C], "c_wk_f")
        wv_f = cload(wv_in, [C, 2 * C], "c_wv_f")
        wp_f = cload(wp_in, [C, C], "c_wp_f")

        wq = const.tile([C, C], f32r)
        nc.vector.tensor_copy(wq[:], wq_f[:])
        wk = const.tile([C, C], f32r)
        nc.vector.tensor_copy(wk[:], wk_f[:])
        wv2 = const.tile([C, 2 * C], f32r)
        nc.vector.tensor_copy(wv2[:], wv_f[:])
        wp = const.tile([C, C], f32r)
        nc.vector.tensor_copy(wp[:], wp_f[:])

        ones_c = const.tile([C, 1], f32)
        nc.vector.memset(ones_c[:], 1.0)
        ones_r = const.tile([1, C], f32r)
        nc.vector.memset(ones_r[:].bitcast(f32), 1.0)
        eps_t = const.tile([GROUPS, 1], f32)
        nc.vector.memset(eps_t[:], EPS)

        # ---------------- groupnorm stats (split across DVE and ACT) ----
        st2 = stats.tile([C, 2], f32)
        s2p = stats.tile([C, NCH], f32)
        s1p = stats.tile([C, NCH], f32)
        adum = stats.tile([C, CHW], f32)
        for ch in range(NCH):  # x^2 sums on ACT, x sums on DVE
            sl = slice(ch * CHW, (ch + 1) * CHW)
            nc.scalar.activation(
                adum[:], x_sb[:, sl], AF.Square, accum_out=s2p[:, ch:ch + 1]
            )
            nc.vector.reduce_sum(
                s1p[:, ch:ch + 1], x_sb[:, sl], axis=mybir.AxisListType.X
            )
        nc.vector.reduce_sum(st2[:, 1:2], s2p[:], axis=mybir.AxisListType.X)
        nc.vector.reduce_sum(st2[:, 0:1], s1p[:], axis=mybir.AxisListType.X)

        with tc.tile_pool(name="pa", bufs=2, space="PSUM") as pa:
            gs_ps = pa.tile([GROUPS, 2], f32, tag="qk")
            nc.tensor.matmul(gs_ps[:], ig[:], st2[:], start=True, stop=True)
            gstats = stats.tile([GROUPS, 2], f32)
            nc.vector.tensor_copy(gstats[:], gs_ps[:])
            inv_n = 1.0 / float(GSIZE * HW)
            gmean = stats.tile([GROUPS, 1], f32)
            nc.vector.tensor_scalar_mul(gmean[:], gstats[:, 0:1], inv_n)
            gm2 = stats.tile([GROUPS, 1], f32)
            nc.vector.tensor_scalar_mul(gm2[:], gstats[:, 1:2], inv_n)
            gmsq = stats.tile([GROUPS, 1], f32)
            nc.vector.tensor_mul(gmsq[:], gmean[:], gmean[:])
            gvar = stats.tile([GROUPS, 1], f32)
            nc.vector.tensor_sub(gvar[:], gm2[:], gmsq[:])
            gln = stats.tile([GROUPS, 1], f32)
            nc.scalar.activation(gln[:], gvar[:], AF.Ln, bias=eps_t[:])
            gmr = stats.tile([GROUPS, 2], f32)
            nc.scalar.activation(gmr[:, 1:2], gln[:], AF.Exp, scale=-0.5)
            nc.vector.tensor_copy(gmr[:, 0:1], gmean[:])

            bc_ps = pa.tile([C, 2], f32, tag="qk")
            nc.tensor.matmul(bc_ps[:], igt[:], gmr[:], start=True, stop=True)
            a_c = stats.tile([C, 1], f32)
            b_c = stats.tile([C, 1], f32)
            tmc = stats.tile([C, 1], f32)
            nc.vector.tensor_scalar_mul(a_c[:], gamma[:], bc_ps[:, 1:2])
            nc.vector.tensor_scalar_mul(tmc[:], a_c[:], bc_ps[:, 0:1])
            nc.vector.tensor_sub(b_c[:], beta[:], tmc[:])

            # hn = a_c * x + b_c   (fp32r), per half so QKV can start early
            hn = big.tile([C, HW], f32r, tag="hn")
            q_r = big.tile([C, HW], f32r, tag="q")
            k_r = big.tile([C, HW], f32r, tag="k")
            vt = big.tile([C, NJ, C], f32r, tag="vt")

            def emit_qkv_half(h):
                hs = slice(h * HWID, (h + 1) * HWID)
                nc.scalar.activation(
                    hn[:, hs], x_sb[:, hs], AF.Identity, bias=b_c[:], scale=a_c[:]
                )
                kp = pa.tile([C, HWID], f32, tag="qk")
                for kk in range(4):
                    sl = slice(kk * IBLK, (kk + 1) * IBLK)
                    off = h * HWID + kk * IBLK
                    nc.tensor.matmul(
                        kp[:, sl], wk[:], hn[:, off:off + IBLK], start=True, stop=True
                    )
                # K copy on DVE (keeps ACT free); b_k added via tensor_scalar
                nc.vector.tensor_scalar(
                    k_r[:, hs], kp[:], bk[:], None, mybir.AluOpType.add
                )
                for r in range(2):
                    vp = pa.tile([C, 8, 2 * C], f32, tag="qk")
                    for t in range(8):
                        nt = h * 16 + r * 8 + t
                        nc.tensor.matmul(
                            vp[:, t, :], hn[:, nt * 128:(nt + 1) * 128], wv2[:],
                            start=True, stop=True,
                        )
                    nc.vector.tensor_copy(
                        vt[:, h * 16 + r * 8:h * 16 + (r + 1) * 8, :], vp[:, :, 0:C]
                    )
                qp = pa.tile([C, HWID], f32, tag="qk")
                for kk in range(4):
                    sl = slice(kk * IBLK, (kk + 1) * IBLK)
                    off = h * HWID + kk * IBLK
                    nc.tensor.matmul(
                        qp[:, sl], wq[:], hn[:, off:off + IBLK], start=True, stop=True
                    )
                nc.scalar.activation(q_r[:, hs], qp[:], AF.Identity, bias=bq[:])

            emit_qkv_half(0)
            emit_qkv_half(1)

        # ---------------- main attention loop ----------------
        # PSUM: two independently-released S tiles (2 banks each) + one O
        # accumulator (4 banks). exp is split into two [C, 1024] halves so
        # the S banks free as soon as their half is consumed -> the next
        # iteration's S-matmuls overlap the current exp (keeps PE dense and
        # the HAM clock warm).
        acc = big.tile([C, HW], f32, tag="acc")
        o_sb = big.tile([C, HW], f32, tag="o")
        HQ = HWID // 2  # 1024
        with tc.tile_pool(name="ps_s", bufs=1, space="PSUM") as ps_s, \
             tc.tile_pool(name="ps_o", bufs=1, space="PSUM") as ps_o:
            for h in range(NHALF):
                hsl = slice(h * HWID, (h + 1) * HWID)
                op = ps_o.tile([C, HWID], f32, tag="o_ps")
                for j in range(NJ):
                    pt = ptpool.tile([C, HWID], f32r)
                    for half in range(2):
                        sp = ps_s.tile([C, HQ], f32, tag=f"s{half}")
                        for kk in range(2):
                            qoff = h * HWID + half * HQ + kk * IBLK
                            nc.tensor.matmul(
                                sp[:, kk * IBLK:(kk + 1) * IBLK],
                                k_r[:, j * 128:(j + 1) * 128],
                                q_r[:, qoff:qoff + IBLK],
                                start=True, stop=True,
                            )
                        nc.scalar.activation(
                            pt[:, half * HQ:(half + 1) * HQ], sp[:], AF.Exp
                        )
                    if j == 0:
                        nc.vector.tensor_copy(acc[:, hsl], pt[:].bitcast(f32))
                    else:
                        nc.vector.tensor_add(acc[:, hsl], acc[:, hsl], pt[:].bitcast(f32))
                    for kk in range(4):
                        sl = slice(kk * IBLK, (kk + 1) * IBLK)
                        nc.tensor.matmul(
                            op[:, sl], vt[:, j, :], pt[:, sl],
                            start=(j == 0), stop=(j == NJ - 1),
                        )
                nc.scalar.activation(o_sb[:, hsl], op[:], AF.Copy)

        # ---------------- softmax denominators + projection ----------------
        # DVE reciprocal is ~8 cyc/elem *per lane*; a [1, 4096] layout would
        # put all 4096 on one lane (~34us). Round-trip through a [128, 32]
        # layout via DMA so all lanes work (~0.3us).
        recip = row.tile([1, HW], f32r)
        rs128 = stats.tile([C, HW // C], f32)  # [128, 32]
        rc128 = stats.tile([C, HW // C], f32r)
        with tc.tile_pool(name="prs", bufs=3, space="PSUM") as prs, \
             tc.tile_pool(name="stage", bufs=3) as stg:
            for ib in range(NIB):
                sl = slice(ib * IBLK, (ib + 1) * IBLK)
                rp = prs.tile([1, IBLK], f32)
                nc.tensor.matmul(rp[:], ones_c[:], acc[:, sl], start=True, stop=True)
                st = stg.tile([1, IBLK], f32, tag="stage")
                nc.vector.tensor_copy(st[:], rp[:])
                # scatter [1, 512] -> 16 partitions x 32
                nc.sync.dma_start(rs128[ib * 16:(ib + 1) * 16, :], st[:])
            with nc.allow_low_precision(reason="fp32r rounding of softmax recip is benign"):
                nc.vector.reciprocal(rc128[:], rs128[:])
            nc.sync.dma_start(recip[:], rc128[:])

        o_nrm = big.tile([C, HW], f32r, tag="hn")  # reuses hn slot
        with tc.tile_pool(name="pbc", bufs=2, space="PSUM") as pbc:
            for ib in range(NIB):
                sl = slice(ib * IBLK, (ib + 1) * IBLK)
                bp = pbc.tile([C, IBLK], f32)
                nc.tensor.matmul(bp[:], ones_r[:], recip[:, sl], start=True, stop=True)
                nc.vector.tensor_mul(o_nrm[:, sl], o_sb[:, sl], bp[:])

        out_sb = big.tile([C, HW], f32, tag="scratch")  # reuses xsq slot
        with tc.tile_pool(name="ppj", bufs=2, space="PSUM") as ppj:
            for ib in range(NIB):
                sl = slice(ib * IBLK, (ib + 1) * IBLK)
                pp = ppj.tile([C, IBLK], f32)
                nc.tensor.matmul(pp[:], wp[:], o_nrm[:, sl], start=True, stop=True)
                nc.vector.tensor_scalar(
                    out_sb[:, sl], pp[:], beff[:], None, mybir.AluOpType.add
                )
                nc.vector.tensor_add(out_sb[:, sl], out_sb[:, sl], x_sb[:, sl])
                nc.sync.dma_start(out_dram[:, sl], out_sb[:, sl])

    nc.compile()
    return nc


def _get_nc():
    if "nc" not in _CACHE:
        _CACHE["nc"] = _build()
    return _CACHE["nc"]


def _prep_inputs(x, gamma, beta, w_qkv, b_qkv, w_proj, b_proj):
    x = np.ascontiguousarray(x, dtype=np.float32)
    w_qkv = np.asarray(w_qkv, dtype=np.float32)
    b_qkv = np.asarray(b_qkv, dtype=np.float32)
    w_proj = np.asarray(w_proj, dtype=np.float32)
    b_proj = np.asarray(b_proj, dtype=np.float32)

    wq = w_qkv[0:C, :]
    wk = w_qkv[C:2 * C, :]
    wv = w_qkv[2 * C:3 * C, :]
    bqv = b_qkv[0:C]
    bkv = b_qkv[C:2 * C]
    bvv = b_qkv[2 * C:3 * C]

    wqT = np.ascontiguousarray((wq * SCALE).T)
    wkT = np.ascontiguousarray(wk.T)
    wvT2 = np.concatenate(
        [np.ascontiguousarray(wv.T), np.zeros((C, C), np.float32)], axis=1
    )
    wpT = np.ascontiguousarray(w_proj.T)
    beff = (b_proj + w_proj @ bvv).astype(np.float32)

    ig = np.zeros((C, GROUPS), np.float32)
    ig[np.arange(C), np.arange(C) // GSIZE] = 1.0
    igt = np.ascontiguousarray(ig.T)

    common = {
        "gamma": np.asarray(gamma, np.float32).reshape(C, 1),
        "beta": np.asarray(beta, np.float32).reshape(C, 1),
        "bq": (bqv * SCALE).reshape(C, 1),
        "bk": bkv.reshape(C, 1),
        "beff": beff.reshape(C, 1),
        "wqT": wqT,
        "wkT": wkT,
        "wvT2": np.ascontiguousarray(wvT2),
        "wpT": wpT,
        "ig": ig,
        "igt": igt,
    }
    in_maps = []
    for b in range(B):
        m = dict(common)
        m["x"] = np.ascontiguousarray(x[b].reshape(C, HW))
        in_maps.append(m)
    return in_maps


def kernel(x, gamma, beta, w_qkv, b_qkv, w_proj, b_proj):
    from concourse.bass_utils import run_bass_kernel_spmd

    nc = _get_nc()
    in_maps = _prep_inputs(x, gamma, beta, w_qkv, b_qkv, w_proj, b_proj)
    res = run_bass_kernel_spmd(nc, in_maps, list(range(B)))
    out = np.stack([res.results[b]["out"] for b in range(B)], axis=0)
    return out.reshape(B, C, H, W).astype(np.float32)
